# revision 10
# baseline (speedup 1.0000x reference)
"""Fused cross-attention kernel for TRN2, sharded over 8 NeuronCores.

Sharding: core = 2*b + g  (b = batch 0..3 data-parallel, g = head-group 0..1
tensor-parallel over heads: heads g*8..g*8+7, i.e. columns g*512..(g+1)*512 of
Wq/Wk/Wv and rows g*512..(g+1)*512 of Wo). Each core computes a partial
out = softmax((x@Wq)(ctx@Wk)^T/sqrt(d)) (ctx@Wv) @ Wo_slice for its batch;
the host sums the two head-group partials per batch and adds bo.

On-device layout (per core), all matmul operands bf16, PSUM accum fp32:
  Q^T = (Wq_g)^T x^T    [512, 2048]  (4 sbuf tiles [128, 2048], head-pair per
  K^T = (Wk_g)^T ctx^T  [512, 2048]   tile: head A rows 0-63, head B 64-127)
  V   = ctx @ Wv_g      [2048, 512]  (16 m-tiles [128, 8*65]: per head 64 V
                                      cols + a ones column for softmax sums)
  Attention per head-pair, per n-block(512): S^T tiles [128m, 512n] via
  row-packed K=64 matmuls (2 heads concurrent in the PE array); exp on
  ScalarE (scale=1/8, bias=log-mask[m], bf16 out);
  O^T[65, n] += [V|1]^T @ expS^T accumulated over 16 m-tiles in PSUM
  (row 64 = softmax sums). Normalize: reciprocal_approx_fast of row 64,
  DMA partition-broadcast (0-stride src), DVE multiplies; head B's rows are
  DMA-shifted to partitions 64-127 of the pair O^T tile.
  out = (O^T_norm).T @ Wo_g  via lhsT = O^T_norm. The inner loop is
  software-pipelined: S(t+1) is emitted before attnV(t) so the PE never
  waits on the ScalarE exp of tile t.
"""
import numpy as np

B, N, M = 4, 2048, 2048
DQ = 1024
DC = 1024
H = 16
DH = 64
INNER = 1024
HG = 2            # head groups (tensor parallel)
HPC = H // HG     # heads per core
CI = HPC * DH     # 512 inner dims per core
NCORES = 8
PT = 128          # partition tile
NB = 512          # n-block
KT_DQ = DQ // PT  # 8 contraction tiles for projections
MT = M // PT      # 16 m-tiles
NT = N // PT      # 16 n-tiles
SCALE = DH ** -0.5

_CACHE = {}


def _build_nc():
    import concourse.bass as bass
    import concourse.mybir as mybir
    import concourse.tile as tile
    from concourse import bacc

    F32 = mybir.dt.float32
    BF16 = mybir.dt.bfloat16
    EXP = mybir.ActivationFunctionType.Exp

    nc = bacc.Bacc("TRN2", target_bir_lowering=False, debug=False,
                   num_devices=NCORES)

    xT_d = nc.dram_tensor("xT", [DQ, N], BF16, kind="ExternalInput")
    ctxT_d = nc.dram_tensor("ctxT", [DC, M], BF16, kind="ExternalInput")
    wq_d = nc.dram_tensor("wq", [DQ, CI], BF16, kind="ExternalInput")
    wk_d = nc.dram_tensor("wk", [DC, CI], BF16, kind="ExternalInput")
    wv_d = nc.dram_tensor("wv", [DC, CI], BF16, kind="ExternalInput")
    wo_d = nc.dram_tensor("wo", [CI, INNER], BF16, kind="ExternalInput")
    mb_d = nc.dram_tensor("maskb", [MT, PT], F32, kind="ExternalInput")
    out_d = nc.dram_tensor("out", [N, INNER], F32, kind="ExternalOutput")

    with tile.TileContext(nc) as tc:
      with tc.tile_pool(name="persist", bufs=1) as pp:
        kt = [pp.tile([PT, M], BF16, tag=f"kt{p}", name=f"kt{p}")
              for p in range(4)]
        vt = [pp.tile([PT, HPC * (DH + 1)], BF16, tag=f"vt{t}", name=f"vt{t}")
              for t in range(MT)]
        mask_t = pp.tile([PT, MT], F32, tag="mask")
        for t in range(MT):
            nc.sync.dma_start(mask_t[:, t:t + 1], mb_d[t, :])

        # ---------------- Phase A: K^T and V from ctx^T ----------------
        with (
            tc.tile_pool(name="phA", bufs=1) as pa,
            tc.tile_pool(name="phA_s", bufs=2) as pas,
            tc.tile_pool(name="psA", bufs=4, space="PSUM") as psA,
        ):
            wk_t = [pa.tile([PT, CI], BF16, tag=f"wk{k}", name=f"wk{k}")
                    for k in range(KT_DQ)]
            wv_t = [pa.tile([PT, CI], BF16, tag=f"wv{k}", name=f"wv{k}")
                    for k in range(KT_DQ)]
            for k in range(KT_DQ):
                nc.sync.dma_start(wk_t[k][:], wk_d[k * PT:(k + 1) * PT, :])
                nc.sync.dma_start(wv_t[k][:], wv_d[k * PT:(k + 1) * PT, :])
            for q in range(M // NB):   # stream ctx^T in m-quarters
                mq = slice(q * NB, (q + 1) * NB)
                ctx = []
                for k in range(KT_DQ):
                    c = pas.tile([PT, NB], BF16, tag=f"ctx{k}", name=f"ctx{k}")
                    nc.sync.dma_start(c[:], ctxT_d[k * PT:(k + 1) * PT, mq])
                    ctx.append(c)
                for p in range(4):
                    ps = psA.tile([PT, NB], F32, tag="psA")
                    for k in range(KT_DQ):
                        nc.tensor.matmul(
                            ps[:], wk_t[k][:, p * PT:(p + 1) * PT],
                            ctx[k][:],
                            start=(k == 0), stop=(k == KT_DQ - 1))
                    nc.vector.tensor_copy(kt[p][:, mq], ps[:])
                for ti in range(NB // PT):
                    t = q * (NB // PT) + ti
                    ps = psA.tile([PT, CI], F32, tag="psA")
                    for k in range(KT_DQ):
                        nc.tensor.matmul(
                            ps[:], ctx[k][:, ti * PT:(ti + 1) * PT],
                            wv_t[k][:],
                            start=(k == 0), stop=(k == KT_DQ - 1))
                    dst = vt[t][:].rearrange("p (h c) -> p h c", c=DH + 1)
                    nc.vector.tensor_copy(
                        dst[:, :, 0:DH],
                        ps[:].rearrange("p (h c) -> p h c", c=DH))
                    nc.vector.memset(dst[:, :, DH:DH + 1], 1.0)

        # ---------------- Phase B: Q^T from x^T ----------------
        with tc.tile_pool(name="qt_pool", bufs=1) as pq:
            qt = [pq.tile([PT, N], BF16, tag=f"qt{p}", name=f"qt{p}")
                  for p in range(4)]
            with (
                tc.tile_pool(name="phB", bufs=1) as pb,
                tc.tile_pool(name="phB_s", bufs=2) as pbs,
                tc.tile_pool(name="psB", bufs=4, space="PSUM") as psB,
            ):
                wq_t = [pb.tile([PT, CI], BF16, tag=f"wq{k}", name=f"wq{k}")
                        for k in range(KT_DQ)]
                for k in range(KT_DQ):
                    nc.sync.dma_start(wq_t[k][:], wq_d[k * PT:(k + 1) * PT, :])
                for q in range(N // NB):
                    nq = slice(q * NB, (q + 1) * NB)
                    xt = []
                    for k in range(KT_DQ):
                        c = pbs.tile([PT, NB], BF16, tag=f"xt{k}",
                                     name=f"xt{k}")
                        nc.sync.dma_start(c[:], xT_d[k * PT:(k + 1) * PT, nq])
                        xt.append(c)
                    for p in range(4):
                        ps = psB.tile([PT, NB], F32, tag="psB")
                        for k in range(KT_DQ):
                            nc.tensor.matmul(
                                ps[:], wq_t[k][:, p * PT:(p + 1) * PT],
                                xt[k][:],
                                start=(k == 0), stop=(k == KT_DQ - 1))
                        nc.vector.tensor_copy(qt[p][:, nq], ps[:])

            # ---------------- Phase C: attention ----------------
            with tc.tile_pool(name="ot_pool", bufs=1) as po:
                ot = [po.tile([PT, N], BF16, tag=f"ot{p}", name=f"ot{p}")
                      for p in range(4)]

                with (
                    tc.tile_pool(name="attn_sb", bufs=3) as asb,
                    tc.tile_pool(name="attn_small", bufs=2) as asmall,
                    tc.tile_pool(name="ps_s", bufs=2, space="PSUM") as ps_s,
                    tc.tile_pool(name="ps_o", bufs=2, space="PSUM") as ps_o,
                ):
                    def emit_normalize(prev):
                        p, jq, oA, oB = prev
                        # plain copies handle the partition shift 64->0;
                        # reciprocal_approx_fast must stay partition-aligned
                        # (shifted input breaks its custom-DVE ucode on HW)
                        sums = asmall.tile([1, 2 * NB], F32, tag="sums",
                                           name="sums")
                        nc.vector.tensor_copy(sums[0:1, 0:NB],
                                              oA[DH:DH + 1, :])
                        nc.vector.tensor_copy(sums[0:1, NB:2 * NB],
                                              oB[DH:DH + 1, :])
                        rr = asmall.tile([1, 2 * NB], F32, tag="rr",
                                         name="rr")
                        nc.vector.reciprocal_approx_fast(
                            rr[0:1, :], sums[0:1, :])
                        # broadcast the recip row down to 64 partitions
                        bcs = asmall.tile([DH, 2 * NB], F32, tag="bcs",
                                          name="bcs")
                        nc.gpsimd.partition_broadcast(
                            bcs[:], rr[0:1, :])
                        nc.vector.tensor_mul(
                            ot[p][0:DH, jq], oA[0:DH, :], bcs[:, 0:NB])
                        tmpB = asmall.tile([DH, NB], BF16, tag="tmpB",
                                           name="tmpB")
                        nc.vector.tensor_mul(
                            tmpB[:], oB[0:DH, :], bcs[:, NB:2 * NB])
                        nc.sync.dma_start(ot[p][DH:2 * DH, jq], tmpB[:])

                    def emit_s(p, jq, t):
                        sps = ps_s.tile([PT, 2 * NB], F32, tag="sps",
                                        name="sps")
                        nc.tensor.matmul(
                            sps[:, 0:NB],
                            kt[p][0:DH, t * PT:(t + 1) * PT],
                            qt[p][0:DH, jq], start=True, stop=True)
                        nc.tensor.matmul(
                            sps[:, NB:2 * NB],
                            kt[p][DH:2 * DH, t * PT:(t + 1) * PT],
                            qt[p][DH:2 * DH, jq], start=True, stop=True)
                        pe = asb.tile([PT, 2 * NB], BF16, tag="pe", name="pe")
                        nc.scalar.activation(pe[:], sps[:], EXP,
                                             bias=mask_t[:, t:t + 1],
                                             scale=SCALE)
                        return pe

                    def emit_av(pes, oA, oB, hA, hB, t):
                        nc.tensor.matmul(
                            oA[:],
                            vt[t][:, hA * (DH + 1):(hA + 1) * (DH + 1)],
                            pes[:, 0:NB],
                            start=(t == 0), stop=(t == MT - 1))
                        nc.tensor.matmul(
                            oB[:],
                            vt[t][:, hB * (DH + 1):(hB + 1) * (DH + 1)],
                            pes[:, NB:2 * NB],
                            start=(t == 0), stop=(t == MT - 1))

                    prev = None
                    for p in range(4):
                        hA, hB = 2 * p, 2 * p + 1
                        for j in range(N // NB):
                            jq = slice(j * NB, (j + 1) * NB)
                            oA = ps_o.tile([DH + 1, NB], F32, tag="oA",
                                           name="oA")
                            oB = ps_o.tile([DH + 1, NB], F32, tag="oB",
                                           name="oB")
                            pes = [None] * MT
                            for t in range(MT):
                                pes[t] = emit_s(p, jq, t)
                                if t == 2 and prev is not None:
                                    emit_normalize(prev)
                                    prev = None
                                if t >= 1:
                                    emit_av(pes[t - 1], oA, oB, hA, hB, t - 1)
                                    pes[t - 1] = None
                            emit_av(pes[MT - 1], oA, oB, hA, hB, MT - 1)
                            prev = (p, jq, oA, oB)
                    emit_normalize(prev)

                # ---------------- Phase D: out = O^T.T @ Wo ----------------
                with (
                    tc.tile_pool(name="phD", bufs=1) as pd,
                    tc.tile_pool(name="phD_out", bufs=3) as pdo,
                    tc.tile_pool(name="psD", bufs=4, space="PSUM") as psD,
                ):
                    wo_t = [pd.tile([PT, INNER], BF16, tag=f"wo{k}",
                                    name=f"wo{k}") for k in range(4)]
                    for k in range(4):
                        nc.sync.dma_start(wo_t[k][:],
                                          wo_d[k * PT:(k + 1) * PT, :])
                    for nt in range(NT):
                        for c in range(INNER // NB):
                            ps = psD.tile([PT, NB], F32, tag="psD")
                            for k in range(4):
                                nc.tensor.matmul(
                                    ps[:],
                                    ot[k][:, nt * PT:(nt + 1) * PT],
                                    wo_t[k][:, c * NB:(c + 1) * NB],
                                    start=(k == 0), stop=(k == 3))
                            ob = pdo.tile([PT, NB], F32, tag="ob")
                            nc.vector.tensor_copy(ob[:], ps[:])
                            nc.sync.dma_start(
                                out_d[nt * PT:(nt + 1) * PT,
                                      c * NB:(c + 1) * NB],
                                ob[:])

    nc.compile()
    return nc


def _get_nc():
    if "nc" not in _CACHE:
        _CACHE["nc"] = _build_nc()
    return _CACHE["nc"]


def make_in_maps(x, context, mask, Wq, Wk, Wv, Wo):
    import ml_dtypes
    bf16 = ml_dtypes.bfloat16
    x = np.asarray(x, np.float32)
    context = np.asarray(context, np.float32)
    mask = np.asarray(mask)
    maskb = np.where(mask, np.float32(0.0),
                     np.float32(-1e30)).astype(np.float32)
    wqs, wks, wvs, wos = [], [], [], []
    for g in range(HG):
        cs = slice(g * CI, (g + 1) * CI)
        wqs.append(np.ascontiguousarray(
            np.asarray(Wq, np.float32)[:, cs].astype(bf16)))
        wks.append(np.ascontiguousarray(
            np.asarray(Wk, np.float32)[:, cs].astype(bf16)))
        wvs.append(np.ascontiguousarray(
            np.asarray(Wv, np.float32)[:, cs].astype(bf16)))
        wos.append(np.ascontiguousarray(
            np.asarray(Wo, np.float32)[cs, :].astype(bf16)))
    in_maps = []
    for b in range(B):
        xT = np.ascontiguousarray(x[b].T.astype(bf16))
        ctxT = np.ascontiguousarray(context[b].T.astype(bf16))
        mb = np.ascontiguousarray(maskb[b].reshape(MT, PT))
        for g in range(HG):
            in_maps.append({
                "xT": xT, "ctxT": ctxT,
                "wq": wqs[g], "wk": wks[g], "wv": wvs[g], "wo": wos[g],
                "maskb": mb,
            })
    return in_maps


def combine(results, bo):
    bo = np.asarray(bo, np.float32)
    out = np.empty((B, N, INNER), np.float32)
    for b in range(B):
        out[b] = (results[2 * b]["out"] + results[2 * b + 1]["out"]
                  + bo[None, :])
    return out


def kernel(x, context, mask, Wq, Wk, Wv, Wo, bo):
    from concourse import bass2jax
    nc = _get_nc()
    in_maps = make_in_maps(x, context, mask, Wq, Wk, Wv, Wo)
    results = bass2jax.run_bass_via_pjrt(nc, in_maps, n_cores=NCORES)
    return combine(results, bo)
